# revision 37
# baseline (speedup 1.0000x reference)
"""Bass/Trainium2 kernel for nn_BiLSTM_9028021256417.

Reference computation: 2-layer "bidirectional" LSTM where the fw and bw
chains are independent (no concat between layers), residual add on the
last layer, final output = (fw + bw) / 2.

Sharding (8 NeuronCores, SPMD — identical program, per-core data):
  cores 0-3: forward direction,  batch shards of 128
  cores 4-7: backward direction, batch shards of 128 (host feeds
             time-reversed x, so the device program is direction-agnostic)

Device layout: all state transposed —
  h, c           : [H=128 partitions, B=128 free]
  PSUM gate bank : [128, 4*B] gate order (g, f, i, o) along free dim
  per-gate matmul: out[128, B] (+)= lhsT(W_g|U_g [128,128]).T @ rhs(x_t^T|h)

v2 (lean-cycle) design, from baseline trace analysis:
  - The kernel is latency-bound on the per-step recurrence cycle
    (z0 -> gates -> c -> tanh -> h -> U0 matmuls -> z0'), not engine
    throughput. So: optimize the cycle.
  - Native Tanh for the candidate gate (Sigmoid+Tanh share one ACT
    table: the baseline trace shows a single ACT_TABLE_LOAD), killing
    the 2*sigmoid(2x)-1 rescale op and its K=1 bias-fix matmuls.
  - Per-gate PSUM accumulation groups + g-gate-first U-matmul order:
    tanh(zg) starts after the FIRST U matmul lands, sigmoid(f,i,o)
    after the fourth.
  - Layer 1 runs one step behind layer 0 ("deep skew"): z1(t-1) is
    fully accumulated by the start of iteration t, so its ACT work
    fills the gap while layer 0's DVE chain runs, instead of colliding
    with layer 0's critical tanh(c) as in the baseline.
  - DVE queue ordered so the h-muls are never stuck behind the out-add.
  - x loads batched 4 steps/DMA, out stores 2 steps/DMA.
"""

import numpy as np
import ml_dtypes

import concourse.bass as bass
import concourse.tile as tile
from concourse import bacc, mybir
from concourse.bass_utils import run_bass_kernel_spmd

AF = mybir.ActivationFunctionType
FP32 = mybir.dt.float32
BF16 = mybir.dt.bfloat16
NP_BF16 = ml_dtypes.bfloat16

# Problem sizes (hardcoded per the harness contract).
B_TOT, T, E, H = 512, 200, 128, 128
NCORES = 8
NSHARD = 4          # batch shards per direction
B = B_TOT // NSHARD  # 128 per core
P = 128
NG = 4
XCHUNK = 4          # x timesteps per input DMA
OCHUNK = 2          # out timesteps per output DMA

# Device gate order (g, f, i, o) -> Keras 4H order is (i, f, g, o).
KERAS_IDX = [2, 1, 0, 3]  # g, f, i, o
COL_G = slice(0 * B, 1 * B)
COL_F = slice(1 * B, 2 * B)
COL_I = slice(2 * B, 3 * B)
COL_O = slice(3 * B, 4 * B)
COL_FIO = slice(1 * B, 4 * B)


def _build_program(scalar_bias: float | None, t_steps: int = T):
    """Build the SPMD per-core Bass program (see module docstring)."""
    nc = bacc.Bacc("TRN2", target_bir_lowering=False, debug=False)

    # x chunked host-side as [T/XCHUNK, E, XCHUNK*B]; out written chunked
    # as [T/OCHUNK, H, OCHUNK*B] — so batched DMAs map 1:1 onto 2-d tiles.
    assert t_steps % XCHUNK == 0 and t_steps % OCHUNK == 0
    xT = nc.dram_tensor(
        "xT", [t_steps // XCHUNK, E, XCHUNK * B], BF16, kind="ExternalInput"
    ).ap()
    w = nc.dram_tensor("w", [2, NG, P, P], BF16, kind="ExternalInput").ap()
    u = nc.dram_tensor("u", [2, NG, P, P], BF16, kind="ExternalInput").ap()
    bias = nc.dram_tensor("bias", [2, NG, P, 1], FP32, kind="ExternalInput").ap()
    out = nc.dram_tensor(
        "out", [t_steps // OCHUNK, H, OCHUNK * B], FP32, kind="ExternalOutput"
    ).ap()

    with tile.TileContext(nc) as tc:
        with (
            tc.tile_pool(name="wpool", bufs=1) as wpool,
            tc.tile_pool(name="xpool", bufs=5) as xpool,
            tc.tile_pool(name="zg0pool", bufs=2, space="PSUM") as zg0pool,
            tc.tile_pool(name="z0pool", bufs=2, space="PSUM") as z0pool,
            tc.tile_pool(name="z1pool", bufs=2, space="PSUM") as z1pool,
            tc.tile_pool(name="gpool", bufs=3) as gpool,
            tc.tile_pool(name="tpool", bufs=3) as tpool,
            tc.tile_pool(name="cpool", bufs=3) as cpool,
            tc.tile_pool(name="hpool", bufs=4) as hpool,
            tc.tile_pool(name="opool", bufs=4) as opool,
        ):
            w_t: dict = {}
            u_t: dict = {}
            b_t: dict = {}
            for l in range(2):
                for g in range(NG):
                    wt = wpool.tile([P, P], BF16, tag=f"w{l}{g}")
                    nc.sync.dma_start(wt[:], w[l, g])
                    w_t[l, g] = wt
                    ut = wpool.tile([P, P], BF16, tag=f"u{l}{g}")
                    nc.sync.dma_start(ut[:], u[l, g])
                    u_t[l, g] = ut
                    if scalar_bias is None:
                        bt = wpool.tile([P, 1], FP32, tag=f"b{l}{g}")
                        nc.sync.dma_start(bt[:], bias[l, g])
                        b_t[l, g] = bt

            def bias_for(l, g):
                if scalar_bias is not None:
                    return float(scalar_bias)
                return b_t[l, g][:]

            # Layer-1 sigmoid-trick bias fix (scalar-bias fast path): its
            # g-gate is computed as 2*sigmoid(2*zg)-1 with host-doubled
            # weights, so it needs bias 2s while the single fused sigmoid
            # applies s; add the missing +s via a K=1 rank-1 matmul.
            if scalar_bias is not None:
                fix_lhs = wpool.tile([1, P], BF16, tag="fix_lhs")
                nc.vector.memset(fix_lhs[:], float(scalar_bias))
                fix_rhs = wpool.tile([1, B], BF16, tag="fix_rhs")
                nc.vector.memset(fix_rhs[:], 1.0)

            xtiles: dict = {}

            def load_x(t0):
                """DMA the XCHUNK-step x chunk starting at t0 into SBUF."""
                assert t0 % XCHUNK == 0
                xt = xpool.tile([P, XCHUNK * B], BF16, tag="xt")
                nc.sync.dma_start(xt[:], xT[t0 // XCHUNK])
                for k in range(XCHUNK):
                    xtiles[t0 + k] = (xt, k)

            def emit_x(t):
                """x-projection matmuls for step t. The g gate gets its
                OWN PSUM bank (own accumulation group, closed by U0_g
                alone) so tanh(zg) starts after the first U matmul; the
                f,i,o bank's group closes at the last U0 matmul. NOTE:
                concurrently-open groups must live in different banks —
                interleaved open groups within one bank corrupt PSUM.
                At t=0 there are no U0 matmuls (h(-1)=0): close here."""
                xt, k = xtiles.pop(t)
                rhs = xt[:, k * B : (k + 1) * B]
                zg = zg0pool.tile([P, NG * B], FP32, tag="zg0")
                nc.tensor.matmul(
                    zg[:, 0:B], lhsT=w_t[0, 0][:], rhs=rhs,
                    start=True, stop=(t == 0),
                )
                z0 = z0pool.tile([P, NG * B], FP32, tag="z0")
                for g in range(1, NG):
                    nc.tensor.matmul(
                        z0[:, g * B : (g + 1) * B],
                        lhsT=w_t[0, g][:], rhs=rhs,
                        start=(g == 1), stop=(t == 0 and g == NG - 1),
                    )
                return (zg, z0)

            def emit_u0(z0pair, h0_prev):
                """Recurrent matmuls; g first (closes its own bank's
                group, unblocking tanh_g immediately)."""
                zg, z0 = z0pair
                nc.tensor.matmul(
                    zg[:, 0:B], lhsT=u_t[0, 0][:], rhs=h0_prev[:],
                    start=False, stop=True,
                )
                for g in range(1, NG):
                    nc.tensor.matmul(
                        z0[:, g * B : (g + 1) * B],
                        lhsT=u_t[0, g][:], rhs=h0_prev[:],
                        start=False, stop=(g == NG - 1),
                    )

            def emit_w1_open(h0, close: bool):
                """Open z1(t) with W1 @ h0(t). close=True when there is
                no U1 term (first step: h1(-1) = 0)."""
                z1 = z1pool.tile([P, NG * B], FP32, tag="z1")
                for g in range(NG):
                    nc.tensor.matmul(
                        z1[:, g * B : (g + 1) * B],
                        lhsT=w_t[1, g][:], rhs=h0[:],
                        start=(g == 0),
                        stop=(close and scalar_bias is None and g == NG - 1),
                    )
                if scalar_bias is not None:
                    nc.tensor.matmul(
                        z1[:, COL_G], lhsT=fix_lhs[:], rhs=fix_rhs[:],
                        start=False, stop=close,
                    )
                return z1

            def emit_u1_close(z1, h1_prev):
                """Close z1(t) with U1 @ h1(t-1); g first."""
                for g in range(NG):
                    nc.tensor.matmul(
                        z1[:, g * B : (g + 1) * B],
                        lhsT=u_t[1, g][:], rhs=h1_prev[:],
                        start=False, stop=(g == NG - 1),
                    )

            def gates_l0(z0pair):
                """Layer 0: g = tanh(zg + b_g) from its own bank (native
                Tanh — same ACT table as Sigmoid, no table reload), then
                f,i,o = sigmoid(z + b)."""
                zg, z0 = z0pair
                ys = gpool.tile([P, NG * B], BF16, tag="ys0")
                nc.scalar.activation(ys[:, COL_G], zg[:, 0:B],
                                     AF.Tanh, bias=bias_for(0, 0))
                if scalar_bias is not None:
                    nc.scalar.activation(ys[:, COL_FIO], z0[:, COL_FIO],
                                         AF.Sigmoid, bias=float(scalar_bias))
                else:
                    for g in range(1, NG):
                        nc.scalar.activation(
                            ys[:, g * B : (g + 1) * B],
                            z0[:, g * B : (g + 1) * B],
                            AF.Sigmoid, bias=bias_for(0, g),
                        )
                return ys

            def gates_l1(z1s):
                """Layer 1: sigmoid-trick — ONE fused sigmoid over all 4
                gates (g-gate weights host-doubled, +s fix matmul), then
                s = 2*sig-1 on DVE. Falls back to per-gate ops when the
                bias is not uniform."""
                ys = gpool.tile([P, NG * B], BF16, tag="ys1")
                if scalar_bias is not None:
                    nc.scalar.activation(ys[:, :], z1s[:, :],
                                         AF.Sigmoid, bias=float(scalar_bias))
                    s = tpool.tile([P, B], BF16, tag="s1")
                    # GpSimd: keeps layer-1's rescale off the DVE queue,
                    # where the scheduler parks it in front of the
                    # cycle-critical h0 multiply.
                    nc.gpsimd.tensor_scalar(
                        s[:], ys[:, COL_G], 2.0, -1.0,
                        mybir.AluOpType.mult, mybir.AluOpType.add,
                    )
                else:
                    nc.scalar.activation(ys[:, COL_G], z1s[:, COL_G],
                                         AF.Tanh, bias=bias_for(1, 0))
                    for g in range(1, NG):
                        nc.scalar.activation(
                            ys[:, g * B : (g + 1) * B],
                            z1s[:, g * B : (g + 1) * B],
                            AF.Sigmoid, bias=bias_for(1, g),
                        )
                    s = None
                return ys, s

            def emit_t1(l, ys, g_ap=None):
                # bf16 out keeps the DVE 2x perf mode (all operands 2-byte);
                # t1 = i*g is in (-1,1) and its inputs are already bf16.
                # Layer 1's product runs on GpSimd (slack engine) so the
                # DVE queue holds nothing ahead of the critical h0 mul.
                t1 = tpool.tile([P, B], BF16, tag=f"t1{l}")
                g_in = g_ap if g_ap is not None else ys[:, COL_G]
                eng = nc.gpsimd if l == 1 else nc.vector
                eng.tensor_mul(t1[:], ys[:, COL_I], g_in)
                return t1

            def emit_tfc(l, ys, c_prev):
                t2 = tpool.tile([P, B], FP32, tag=f"t2{l}")
                nc.gpsimd.tensor_mul(t2[:], ys[:, COL_F], c_prev[:])
                return t2

            def emit_c(l, t1, t2):
                c_new = cpool.tile([P, B], FP32, tag=f"c{l}")
                if t2 is None:
                    # first step: c = i*g; materialize as fp32 (GpSimd's
                    # f*c next step expects an fp32 c operand)
                    nc.vector.tensor_copy(c_new[:], t1[:])
                else:
                    nc.vector.tensor_add(c_new[:], t1[:], t2[:])
                return c_new

            def emit_tanh_c(l, c_new):
                tch = gpool.tile([P, B], BF16, tag=f"tc{l}")
                nc.scalar.activation(tch[:], c_new[:], AF.Tanh)
                return tch

            def emit_h(l, ys, tch):
                h_new = hpool.tile([P, B], BF16, tag=f"h{l}")
                nc.vector.tensor_mul(h_new[:], ys[:, COL_O], tch[:])
                return h_new

            ostage: dict = {}
            opending: list = []

            def flush_out():
                """Issue deferred out-DMAs (deps completed last iteration,
                so the GpSimd queue never head-of-line blocks on them)."""
                while opending:
                    row, ot = opending.pop(0)
                    # GpSimd queue: keeps stores off the Sync queue, whose
                    # head-of-line x-DMA WAR waits would delay them (and
                    # cascade into a DVE stall via the staging-tile WAR).
                    nc.gpsimd.dma_start(out[row], ot[:])

            def emit_out(t, h1t, h0t):
                """out(t) = h1(t) + h0(t), staged; DMA every OCHUNK."""
                base = (t // OCHUNK) * OCHUNK
                if t == base:
                    ostage[base] = opool.tile(
                        [P, OCHUNK * B], FP32, tag="ot", name="ot"
                    )
                ot = ostage[base]
                k = t - base
                nc.vector.tensor_add(ot[:, k * B : (k + 1) * B], h1t[:], h0t[:])
                if k == OCHUNK - 1:
                    opending.append((base // OCHUNK, ot))
                    del ostage[base]

            def l1_head(z1s, c1p):
                """Layer-1 gates + products for a step, given closed z1."""
                ys1, s1 = gates_l1(z1s)
                t1_1 = emit_t1(1, ys1, g_ap=(s1[:] if s1 is not None else None))
                tfc_1 = emit_tfc(1, ys1, c1p) if c1p is not None else None
                return ys1, t1_1, tfc_1

            # ---- software pipeline -------------------------------------
            # Iteration t computes layer-0 step t and layer-1 step t-1.
            c0_prev = None           # c0(t-1)
            c1_prev = None           # c1(t-2)
            h0_for_out: dict = {}    # h0(s) kept until out(s)
            z1_prev = None           # z1(t-1), closed by end of iter t-1

            load_x(0)
            if t_steps > XCHUNK:
                load_x(XCHUNK)
            z0 = emit_x(0)

            for t in range(t_steps):
                flush_out()
                # PE: dep-free x-projection for t+1 first
                if t + 1 < t_steps:
                    nxt = t + 1 + XCHUNK
                    if (t + 1) % XCHUNK == 0 and nxt < t_steps:
                        load_x(nxt)
                    z0_next = emit_x(t + 1)
                else:
                    z0_next = None

                # --- layer 0, step t: critical chain head (ACT)
                ys0 = gates_l0(z0)
                t1_0 = emit_t1(0, ys0)
                tfc_0 = emit_tfc(0, ys0, c0_prev) if c0_prev is not None else None

                # --- layer 1, step t-1: ACT gap-fillers (z1(t-1) ready)
                if z1_prev is not None:
                    ys1, t1_1, tfc_1 = l1_head(z1_prev, c1_prev)
                else:
                    ys1 = None

                # --- layer 0 tail: c, tanh, h (the critical cycle)
                c0 = emit_c(0, t1_0, tfc_0)
                tc0 = emit_tanh_c(0, c0)
                h0 = emit_h(0, ys0, tc0)
                h0_for_out[t] = h0

                # --- PE: recurrent matmuls right behind h0
                if z0_next is not None:
                    emit_u0(z0_next, h0)
                z1 = emit_w1_open(h0, close=(t == 0))  # h1(-1) = 0 at t=0

                # --- layer 1 tail for step t-1: c1, tanh, h1, out, U1
                if ys1 is not None:
                    c1 = emit_c(1, t1_1, tfc_1)
                    tc1 = emit_tanh_c(1, c1)
                    h1 = emit_h(1, ys1, tc1)
                    emit_out(t - 1, h1, h0_for_out.pop(t - 1))
                    emit_u1_close(z1, h1)
                    c1_prev = c1

                c0_prev = c0
                z1_prev = z1
                z0 = z0_next

            # ---- epilogue: layer-1 step T-1 ----------------------------
            ys1, t1_1, tfc_1 = l1_head(z1_prev, c1_prev)
            c1 = emit_c(1, t1_1, tfc_1)
            tc1 = emit_tanh_c(1, c1)
            h1 = emit_h(1, ys1, tc1)
            emit_out(t_steps - 1, h1, h0_for_out.pop(t_steps - 1))
            flush_out()

    nc.compile()
    return nc


_PROGRAM_CACHE: dict = {}


def _get_program(scalar_bias, t_steps: int = T):
    key = (scalar_bias, t_steps)
    if key not in _PROGRAM_CACHE:
        _PROGRAM_CACHE[key] = _build_program(scalar_bias, t_steps)
    return _PROGRAM_CACHE[key]


def _prep_inputs(x, W, U, b, scalar_bias):
    """Build the 8 per-core input maps."""
    in_maps = []
    per_dir = {}
    for d in range(2):
        wd = np.empty((2, NG, P, P), dtype=NP_BF16)
        ud = np.empty((2, NG, P, P), dtype=NP_BF16)
        bd = np.empty((2, NG, P, 1), dtype=np.float32)
        for l in range(2):
            for g in range(NG):
                ks = KERAS_IDX[g]
                # layer-1 candidate gate uses the sigmoid trick
                # tanh(z) = 2*sigmoid(2z) - 1: double its weights
                # (fast path only; +s bias fix is a device matmul)
                sc = 2.0 if (l == 1 and g == 0 and scalar_bias is not None) else 1.0
                wd[l, g] = (sc * W[l, d][:, ks * H : (ks + 1) * H]).astype(NP_BF16)
                ud[l, g] = (sc * U[l, d][:, ks * H : (ks + 1) * H]).astype(NP_BF16)
                bd[l, g, :, 0] = b[l, d][ks * H : (ks + 1) * H].astype(np.float32)
        per_dir[d] = (wd, ud, bd)

    for core in range(NCORES):
        d = core // NSHARD
        s = core % NSHARD
        xs = x[s * B : (s + 1) * B]           # [B, T, E]
        if d == 1:
            xs = xs[:, ::-1, :]               # time-reverse for backward dir
        xTc = np.transpose(xs, (1, 2, 0))     # [T, E, B]
        # chunk: [T/XC, XC, E, B] -> [T/XC, E, XC, B] -> [T/XC, E, XC*B]
        xTc = np.transpose(
            xTc.reshape(T // XCHUNK, XCHUNK, E, B), (0, 2, 1, 3)
        ).reshape(T // XCHUNK, E, XCHUNK * B)
        xTc = np.ascontiguousarray(xTc).astype(NP_BF16)
        wd, ud, bd = per_dir[d]
        in_maps.append({"xT": xTc, "w": wd, "u": ud, "bias": bd})
    return in_maps


def _unchunk_out(o):
    """[T/OC, H, OC*B] -> [T, H, B]"""
    o = o.reshape(T // OCHUNK, H, OCHUNK, B)
    return np.transpose(o, (0, 2, 1, 3)).reshape(T, H, B)


def _postprocess(results, dtype):
    full = np.empty((B_TOT, T, H), dtype=np.float32)
    for s in range(NSHARD):
        fw = _unchunk_out(np.asarray(results[s]["out"]))           # [T, H, B]
        bw = _unchunk_out(np.asarray(results[NSHARD + s]["out"]))  # reversed t
        fw_b = np.transpose(fw, (2, 0, 1))            # [B, T, H]
        bw_b = np.transpose(bw, (2, 0, 1))[:, ::-1, :]
        full[s * B : (s + 1) * B] = (fw_b + bw_b) * 0.5
    return full.astype(dtype)


def run(x, W, U, b, **spmd_kwargs):
    """Run the kernel; returns (output, BassKernelResults)."""
    x = np.asarray(x)
    W = np.asarray(W)
    U = np.asarray(U)
    b = np.asarray(b)
    b0 = float(np.asarray(b).flat[0])
    scalar_bias = b0 if np.all(b == b0) else None
    nc = _get_program(scalar_bias)
    in_maps = _prep_inputs(x, W, U, b, scalar_bias)
    res = run_bass_kernel_spmd(nc, in_maps, core_ids=list(range(NCORES)), **spmd_kwargs)
    out = _postprocess(res.results, x.dtype)
    return out, res


def kernel(x, W, U, b):
    out, _ = run(x, W, U, b)
    return out


# revision 39
# speedup vs baseline: 1.0644x; 1.0644x over previous
"""Bass/Trainium2 kernel for nn_BiLSTM_9028021256417.

Reference computation: 2-layer "bidirectional" LSTM where the fw and bw
chains are independent (no concat between layers), residual add on the
last layer, final output = (fw + bw) / 2.

Sharding (8 NeuronCores, SPMD — identical program, per-core data):
  cores 0-3: forward direction,  batch shards of 128
  cores 4-7: backward direction, batch shards of 128 (host feeds
             time-reversed x, so the device program is direction-agnostic)

Device layout: all state transposed —
  h, c           : [H=128 partitions, B=128 free]
  PSUM gate bank : [128, 4*B] gate order (g, f, i, o) along free dim
  per-gate matmul: out[128, B] (+)= lhsT(W_g|U_g [128,128]).T @ rhs(x_t^T|h)

v2 (lean-cycle) design, from baseline trace analysis:
  - The kernel is latency-bound on the per-step recurrence cycle
    (z0 -> gates -> c -> tanh -> h -> U0 matmuls -> z0'), not engine
    throughput. So: optimize the cycle.
  - Native Tanh for the candidate gate (Sigmoid+Tanh share one ACT
    table: the baseline trace shows a single ACT_TABLE_LOAD), killing
    the 2*sigmoid(2x)-1 rescale op and its K=1 bias-fix matmuls.
  - Per-gate PSUM accumulation groups + g-gate-first U-matmul order:
    tanh(zg) starts after the FIRST U matmul lands, sigmoid(f,i,o)
    after the fourth.
  - Layer 1 runs one step behind layer 0 ("deep skew"): z1(t-1) is
    fully accumulated by the start of iteration t, so its ACT work
    fills the gap while layer 0's DVE chain runs, instead of colliding
    with layer 0's critical tanh(c) as in the baseline.
  - DVE queue ordered so the h-muls are never stuck behind the out-add.
  - x loads batched 4 steps/DMA, out stores 2 steps/DMA.
"""

import numpy as np
import ml_dtypes

import concourse.bass as bass
import concourse.tile as tile
from concourse import bacc, mybir
from concourse.bass_utils import run_bass_kernel_spmd

AF = mybir.ActivationFunctionType
FP32 = mybir.dt.float32
BF16 = mybir.dt.bfloat16
NP_BF16 = ml_dtypes.bfloat16

# Problem sizes (hardcoded per the harness contract).
B_TOT, T, E, H = 512, 200, 128, 128
NCORES = 8
NSHARD = 4          # batch shards per direction
B = B_TOT // NSHARD  # 128 per core
P = 128
NG = 4
XCHUNK = 4          # x timesteps per input DMA
OCHUNK = 2          # out timesteps per output DMA

# Device gate order (g, f, i, o) -> Keras 4H order is (i, f, g, o).
KERAS_IDX = [2, 1, 0, 3]  # g, f, i, o
COL_G = slice(0 * B, 1 * B)
COL_F = slice(1 * B, 2 * B)
COL_I = slice(2 * B, 3 * B)
COL_O = slice(3 * B, 4 * B)
COL_FIO = slice(1 * B, 4 * B)


def _build_program(scalar_bias: float | None, t_steps: int = T):
    """Build the SPMD per-core Bass program (see module docstring)."""
    nc = bacc.Bacc("TRN2", target_bir_lowering=False, debug=False)

    # x chunked host-side as [T/XCHUNK, E, XCHUNK*B]; out written chunked
    # as [T/OCHUNK, H, OCHUNK*B] — so batched DMAs map 1:1 onto 2-d tiles.
    assert t_steps % XCHUNK == 0 and t_steps % OCHUNK == 0
    xT = nc.dram_tensor(
        "xT", [t_steps // XCHUNK, E, XCHUNK * B], BF16, kind="ExternalInput"
    ).ap()
    w = nc.dram_tensor("w", [2, NG, P, P], BF16, kind="ExternalInput").ap()
    u = nc.dram_tensor("u", [2, NG, P, P], BF16, kind="ExternalInput").ap()
    bias = nc.dram_tensor("bias", [2, NG, P, 1], FP32, kind="ExternalInput").ap()
    out = nc.dram_tensor(
        "out", [t_steps // OCHUNK, H, OCHUNK * B], FP32, kind="ExternalOutput"
    ).ap()

    with tile.TileContext(nc) as tc:
        with (
            tc.tile_pool(name="wpool", bufs=1) as wpool,
            tc.tile_pool(name="xpool", bufs=5) as xpool,
            tc.tile_pool(name="zg0pool", bufs=2, space="PSUM") as zg0pool,
            tc.tile_pool(name="z0pool", bufs=2, space="PSUM") as z0pool,
            tc.tile_pool(name="z1pool", bufs=2, space="PSUM") as z1pool,
            tc.tile_pool(name="gpool", bufs=3) as gpool,
            tc.tile_pool(name="tpool", bufs=3) as tpool,
            tc.tile_pool(name="cpool", bufs=3) as cpool,
            tc.tile_pool(name="hpool", bufs=4) as hpool,
            tc.tile_pool(name="opool", bufs=4) as opool,
        ):
            w_t: dict = {}
            u_t: dict = {}
            b_t: dict = {}
            for l in range(2):
                for g in range(NG):
                    wt = wpool.tile([P, P], BF16, tag=f"w{l}{g}")
                    nc.sync.dma_start(wt[:], w[l, g])
                    w_t[l, g] = wt
                    ut = wpool.tile([P, P], BF16, tag=f"u{l}{g}")
                    nc.sync.dma_start(ut[:], u[l, g])
                    u_t[l, g] = ut
                    if scalar_bias is None:
                        bt = wpool.tile([P, 1], FP32, tag=f"b{l}{g}")
                        nc.sync.dma_start(bt[:], bias[l, g])
                        b_t[l, g] = bt

            def bias_for(l, g):
                if scalar_bias is not None:
                    return float(scalar_bias)
                return b_t[l, g][:]

            # Layer-1 sigmoid-trick bias fix (scalar-bias fast path): its
            # g-gate is computed as 2*sigmoid(2*zg)-1 with host-doubled
            # weights, so it needs bias 2s while the single fused sigmoid
            # applies s; add the missing +s via a K=1 rank-1 matmul.
            if scalar_bias is not None:
                fix_lhs = wpool.tile([1, P], BF16, tag="fix_lhs")
                nc.vector.memset(fix_lhs[:], float(scalar_bias))
                fix_rhs = wpool.tile([1, B], BF16, tag="fix_rhs")
                nc.vector.memset(fix_rhs[:], 1.0)

            xtiles: dict = {}

            def load_x(t0):
                """DMA the XCHUNK-step x chunk starting at t0 into SBUF."""
                assert t0 % XCHUNK == 0
                xt = xpool.tile([P, XCHUNK * B], BF16, tag="xt")
                nc.sync.dma_start(xt[:], xT[t0 // XCHUNK])
                for k in range(XCHUNK):
                    xtiles[t0 + k] = (xt, k)

            def emit_x(t):
                """x-projection matmuls for step t. The g gate gets its
                OWN PSUM bank (own accumulation group, closed by U0_g
                alone) so tanh(zg) starts after the first U matmul; the
                f,i,o bank's group closes at the last U0 matmul. NOTE:
                concurrently-open groups must live in different banks —
                interleaved open groups within one bank corrupt PSUM.
                At t=0 there are no U0 matmuls (h(-1)=0): close here."""
                xt, k = xtiles.pop(t)
                rhs = xt[:, k * B : (k + 1) * B]
                zg = zg0pool.tile([P, NG * B], FP32, tag="zg0")
                nc.tensor.matmul(
                    zg[:, 0:B], lhsT=w_t[0, 0][:], rhs=rhs,
                    start=True, stop=(t == 0),
                )
                z0 = z0pool.tile([P, NG * B], FP32, tag="z0")
                for g in range(1, NG):
                    nc.tensor.matmul(
                        z0[:, g * B : (g + 1) * B],
                        lhsT=w_t[0, g][:], rhs=rhs,
                        start=(g == 1), stop=(t == 0 and g == NG - 1),
                    )
                return (zg, z0)

            def emit_u0(z0pair, h0_prev):
                """Recurrent matmuls; g first (closes its own bank's
                group, unblocking tanh_g immediately)."""
                zg, z0 = z0pair
                nc.tensor.matmul(
                    zg[:, 0:B], lhsT=u_t[0, 0][:], rhs=h0_prev[:],
                    start=False, stop=True,
                )
                for g in range(1, NG):
                    nc.tensor.matmul(
                        z0[:, g * B : (g + 1) * B],
                        lhsT=u_t[0, g][:], rhs=h0_prev[:],
                        start=False, stop=(g == NG - 1),
                    )

            def emit_w1_open(h0, close: bool):
                """Open z1(t) with W1 @ h0(t). close=True when there is
                no U1 term (first step: h1(-1) = 0)."""
                z1 = z1pool.tile([P, NG * B], FP32, tag="z1")
                for g in range(NG):
                    nc.tensor.matmul(
                        z1[:, g * B : (g + 1) * B],
                        lhsT=w_t[1, g][:], rhs=h0[:],
                        start=(g == 0),
                        stop=(close and scalar_bias is None and g == NG - 1),
                    )
                if scalar_bias is not None:
                    nc.tensor.matmul(
                        z1[:, COL_G], lhsT=fix_lhs[:], rhs=fix_rhs[:],
                        start=False, stop=close,
                    )
                return z1

            def emit_u1_close(z1, h1_prev):
                """Close z1(t) with U1 @ h1(t-1); g first."""
                for g in range(NG):
                    nc.tensor.matmul(
                        z1[:, g * B : (g + 1) * B],
                        lhsT=u_t[1, g][:], rhs=h1_prev[:],
                        start=False, stop=(g == NG - 1),
                    )

            def gates_l0(z0pair):
                """Layer 0: g = tanh(zg + b_g) from its own bank (native
                Tanh — same ACT table as Sigmoid, no table reload), then
                f,i,o = sigmoid(z + b)."""
                zg, z0 = z0pair
                ys = gpool.tile([P, NG * B], BF16, tag="ys0")
                nc.scalar.activation(ys[:, COL_G], zg[:, 0:B],
                                     AF.Tanh, bias=bias_for(0, 0))
                if scalar_bias is not None:
                    nc.scalar.activation(ys[:, COL_FIO], z0[:, COL_FIO],
                                         AF.Sigmoid, bias=float(scalar_bias))
                else:
                    for g in range(1, NG):
                        nc.scalar.activation(
                            ys[:, g * B : (g + 1) * B],
                            z0[:, g * B : (g + 1) * B],
                            AF.Sigmoid, bias=bias_for(0, g),
                        )
                return ys

            def gates_l1(z1s):
                """Layer 1: sigmoid-trick — ONE fused sigmoid over all 4
                gates (g-gate weights host-doubled, +s fix matmul), then
                s = 2*sig-1 on DVE. Falls back to per-gate ops when the
                bias is not uniform."""
                ys = gpool.tile([P, NG * B], BF16, tag="ys1")
                if scalar_bias is not None:
                    nc.scalar.activation(ys[:, :], z1s[:, :],
                                         AF.Sigmoid, bias=float(scalar_bias))
                    s = tpool.tile([P, B], BF16, tag="s1")
                    nc.vector.tensor_scalar(
                        s[:], ys[:, COL_G], 2.0, -1.0,
                        mybir.AluOpType.mult, mybir.AluOpType.add,
                    )
                else:
                    nc.scalar.activation(ys[:, COL_G], z1s[:, COL_G],
                                         AF.Tanh, bias=bias_for(1, 0))
                    for g in range(1, NG):
                        nc.scalar.activation(
                            ys[:, g * B : (g + 1) * B],
                            z1s[:, g * B : (g + 1) * B],
                            AF.Sigmoid, bias=bias_for(1, g),
                        )
                    s = None
                return ys, s

            def emit_t1(l, ys, g_ap=None):
                # bf16 out keeps the DVE 2x perf mode (all operands 2-byte);
                # t1 = i*g is in (-1,1) and its inputs are already bf16.
                t1 = tpool.tile([P, B], BF16, tag=f"t1{l}")
                g_in = g_ap if g_ap is not None else ys[:, COL_G]
                nc.vector.tensor_mul(t1[:], ys[:, COL_I], g_in)
                return t1

            def emit_tfc(l, ys, c_prev):
                t2 = tpool.tile([P, B], FP32, tag=f"t2{l}")
                nc.gpsimd.tensor_mul(t2[:], ys[:, COL_F], c_prev[:])
                return t2

            def emit_c(l, t1, t2):
                c_new = cpool.tile([P, B], FP32, tag=f"c{l}")
                if t2 is None:
                    # first step: c = i*g; materialize as fp32 (GpSimd's
                    # f*c next step expects an fp32 c operand)
                    nc.vector.tensor_copy(c_new[:], t1[:])
                else:
                    nc.vector.tensor_add(c_new[:], t1[:], t2[:])
                return c_new

            def emit_tanh_c(l, c_new):
                tch = gpool.tile([P, B], BF16, tag=f"tc{l}")
                nc.scalar.activation(tch[:], c_new[:], AF.Tanh)
                return tch

            def emit_h(l, ys, tch):
                h_new = hpool.tile([P, B], BF16, tag=f"h{l}")
                nc.vector.tensor_mul(h_new[:], ys[:, COL_O], tch[:])
                return h_new

            ostage: dict = {}
            opending: list = []

            def flush_out():
                """Issue deferred out-DMAs (deps completed last iteration,
                so the GpSimd queue never head-of-line blocks on them)."""
                while opending:
                    row, ot = opending.pop(0)
                    # GpSimd queue: keeps stores off the Sync queue, whose
                    # head-of-line x-DMA WAR waits would delay them (and
                    # cascade into a DVE stall via the staging-tile WAR).
                    nc.gpsimd.dma_start(out[row], ot[:])

            def emit_out(t, h1t, h0t):
                """out(t) = h1(t) + h0(t), staged; DMA every OCHUNK."""
                base = (t // OCHUNK) * OCHUNK
                if t == base:
                    ostage[base] = opool.tile(
                        [P, OCHUNK * B], FP32, tag="ot", name="ot"
                    )
                ot = ostage[base]
                k = t - base
                nc.vector.tensor_add(ot[:, k * B : (k + 1) * B], h1t[:], h0t[:])
                if k == OCHUNK - 1:
                    opending.append((base // OCHUNK, ot))
                    del ostage[base]

            def l1_head(z1s, c1p):
                """Layer-1 gates + products for a step, given closed z1."""
                ys1, s1 = gates_l1(z1s)
                t1_1 = emit_t1(1, ys1, g_ap=(s1[:] if s1 is not None else None))
                tfc_1 = emit_tfc(1, ys1, c1p) if c1p is not None else None
                return ys1, t1_1, tfc_1

            # ---- software pipeline -------------------------------------
            # Iteration t computes layer-0 step t and layer-1 step t-1.
            c0_prev = None           # c0(t-1)
            c1_prev = None           # c1(t-2)
            h0_for_out: dict = {}    # h0(s) kept until out(s)
            z1_prev = None           # z1(t-1), closed by end of iter t-1

            load_x(0)
            if t_steps > XCHUNK:
                load_x(XCHUNK)
            z0 = emit_x(0)

            for t in range(t_steps):
                flush_out()
                # PE: dep-free x-projection for t+1 first
                if t + 1 < t_steps:
                    nxt = t + 1 + XCHUNK
                    if (t + 1) % XCHUNK == 0 and nxt < t_steps:
                        load_x(nxt)
                    z0_next = emit_x(t + 1)
                else:
                    z0_next = None

                # --- layer 0, step t: the full critical chain first.
                # Layer 1's head is emitted AFTER tanh_c0/h0: its DVE/ACT
                # ops are then dependency-gated behind sigma_all1, so
                # nothing can be scheduled ahead of the critical h0 mul
                # on DVE, and tanh_c0 never queues behind sigma_all1.
                ys0 = gates_l0(z0)
                t1_0 = emit_t1(0, ys0)
                tfc_0 = emit_tfc(0, ys0, c0_prev) if c0_prev is not None else None
                c0 = emit_c(0, t1_0, tfc_0)
                tc0 = emit_tanh_c(0, c0)
                h0 = emit_h(0, ys0, tc0)
                h0_for_out[t] = h0

                # --- PE: recurrent matmuls right behind h0
                if z0_next is not None:
                    emit_u0(z0_next, h0)
                z1 = emit_w1_open(h0, close=(t == 0))  # h1(-1) = 0 at t=0

                # --- layer 1, step t-1 (z1(t-1) closed last iteration)
                if z1_prev is not None:
                    ys1, t1_1, tfc_1 = l1_head(z1_prev, c1_prev)
                else:
                    ys1 = None

                # --- layer 1 tail for step t-1: c1, tanh, h1, out, U1
                if ys1 is not None:
                    c1 = emit_c(1, t1_1, tfc_1)
                    tc1 = emit_tanh_c(1, c1)
                    h1 = emit_h(1, ys1, tc1)
                    emit_out(t - 1, h1, h0_for_out.pop(t - 1))
                    emit_u1_close(z1, h1)
                    c1_prev = c1

                c0_prev = c0
                z1_prev = z1
                z0 = z0_next

            # ---- epilogue: layer-1 step T-1 ----------------------------
            ys1, t1_1, tfc_1 = l1_head(z1_prev, c1_prev)
            c1 = emit_c(1, t1_1, tfc_1)
            tc1 = emit_tanh_c(1, c1)
            h1 = emit_h(1, ys1, tc1)
            emit_out(t_steps - 1, h1, h0_for_out.pop(t_steps - 1))
            flush_out()

    nc.compile()
    return nc


_PROGRAM_CACHE: dict = {}


def _get_program(scalar_bias, t_steps: int = T):
    key = (scalar_bias, t_steps)
    if key not in _PROGRAM_CACHE:
        _PROGRAM_CACHE[key] = _build_program(scalar_bias, t_steps)
    return _PROGRAM_CACHE[key]


def _prep_inputs(x, W, U, b, scalar_bias):
    """Build the 8 per-core input maps."""
    in_maps = []
    per_dir = {}
    for d in range(2):
        wd = np.empty((2, NG, P, P), dtype=NP_BF16)
        ud = np.empty((2, NG, P, P), dtype=NP_BF16)
        bd = np.empty((2, NG, P, 1), dtype=np.float32)
        for l in range(2):
            for g in range(NG):
                ks = KERAS_IDX[g]
                # layer-1 candidate gate uses the sigmoid trick
                # tanh(z) = 2*sigmoid(2z) - 1: double its weights
                # (fast path only; +s bias fix is a device matmul)
                sc = 2.0 if (l == 1 and g == 0 and scalar_bias is not None) else 1.0
                wd[l, g] = (sc * W[l, d][:, ks * H : (ks + 1) * H]).astype(NP_BF16)
                ud[l, g] = (sc * U[l, d][:, ks * H : (ks + 1) * H]).astype(NP_BF16)
                bd[l, g, :, 0] = b[l, d][ks * H : (ks + 1) * H].astype(np.float32)
        per_dir[d] = (wd, ud, bd)

    for core in range(NCORES):
        d = core // NSHARD
        s = core % NSHARD
        xs = x[s * B : (s + 1) * B]           # [B, T, E]
        if d == 1:
            xs = xs[:, ::-1, :]               # time-reverse for backward dir
        xTc = np.transpose(xs, (1, 2, 0))     # [T, E, B]
        # chunk: [T/XC, XC, E, B] -> [T/XC, E, XC, B] -> [T/XC, E, XC*B]
        xTc = np.transpose(
            xTc.reshape(T // XCHUNK, XCHUNK, E, B), (0, 2, 1, 3)
        ).reshape(T // XCHUNK, E, XCHUNK * B)
        xTc = np.ascontiguousarray(xTc).astype(NP_BF16)
        wd, ud, bd = per_dir[d]
        in_maps.append({"xT": xTc, "w": wd, "u": ud, "bias": bd})
    return in_maps


def _unchunk_out(o):
    """[T/OC, H, OC*B] -> [T, H, B]"""
    o = o.reshape(T // OCHUNK, H, OCHUNK, B)
    return np.transpose(o, (0, 2, 1, 3)).reshape(T, H, B)


def _postprocess(results, dtype):
    full = np.empty((B_TOT, T, H), dtype=np.float32)
    for s in range(NSHARD):
        fw = _unchunk_out(np.asarray(results[s]["out"]))           # [T, H, B]
        bw = _unchunk_out(np.asarray(results[NSHARD + s]["out"]))  # reversed t
        fw_b = np.transpose(fw, (2, 0, 1))            # [B, T, H]
        bw_b = np.transpose(bw, (2, 0, 1))[:, ::-1, :]
        full[s * B : (s + 1) * B] = (fw_b + bw_b) * 0.5
    return full.astype(dtype)


def run(x, W, U, b, **spmd_kwargs):
    """Run the kernel; returns (output, BassKernelResults)."""
    x = np.asarray(x)
    W = np.asarray(W)
    U = np.asarray(U)
    b = np.asarray(b)
    b0 = float(np.asarray(b).flat[0])
    scalar_bias = b0 if np.all(b == b0) else None
    nc = _get_program(scalar_bias)
    in_maps = _prep_inputs(x, W, U, b, scalar_bias)
    res = run_bass_kernel_spmd(nc, in_maps, core_ids=list(range(NCORES)), **spmd_kwargs)
    out = _postprocess(res.results, x.dtype)
    return out, res


def kernel(x, W, U, b):
    out, _ = run(x, W, U, b)
    return out


# revision 43
# speedup vs baseline: 1.0660x; 1.0016x over previous
"""Bass/Trainium2 kernel for nn_BiLSTM_9028021256417.

Reference computation: 2-layer "bidirectional" LSTM where the fw and bw
chains are independent (no concat between layers), residual add on the
last layer, final output = (fw + bw) / 2.

Sharding (8 NeuronCores, SPMD — identical program, per-core data):
  cores 0-3: forward direction,  batch shards of 128
  cores 4-7: backward direction, batch shards of 128 (host feeds
             time-reversed x, so the device program is direction-agnostic)

Device layout: all state transposed —
  h, c           : [H=128 partitions, B=128 free]
  PSUM gate bank : [128, 4*B] gate order (g, f, i, o) along free dim
  per-gate matmul: out[128, B] (+)= lhsT(W_g|U_g [128,128]).T @ rhs(x_t^T|h)

v2 (lean-cycle) design, from baseline trace analysis:
  - The kernel is latency-bound on the per-step recurrence cycle
    (z0 -> gates -> c -> tanh -> h -> U0 matmuls -> z0'), not engine
    throughput. So: optimize the cycle.
  - Native Tanh for the candidate gate (Sigmoid+Tanh share one ACT
    table: the baseline trace shows a single ACT_TABLE_LOAD), killing
    the 2*sigmoid(2x)-1 rescale op and its K=1 bias-fix matmuls.
  - Per-gate PSUM accumulation groups + g-gate-first U-matmul order:
    tanh(zg) starts after the FIRST U matmul lands, sigmoid(f,i,o)
    after the fourth.
  - Layer 1 runs one step behind layer 0 ("deep skew"): z1(t-1) is
    fully accumulated by the start of iteration t, so its ACT work
    fills the gap while layer 0's DVE chain runs, instead of colliding
    with layer 0's critical tanh(c) as in the baseline.
  - DVE queue ordered so the h-muls are never stuck behind the out-add.
  - x loads batched 4 steps/DMA, out stores 2 steps/DMA.
"""

import numpy as np
import ml_dtypes

import concourse.bass as bass
import concourse.tile as tile
from concourse import bacc, mybir
from concourse.bass_utils import run_bass_kernel_spmd

AF = mybir.ActivationFunctionType
FP32 = mybir.dt.float32
BF16 = mybir.dt.bfloat16
NP_BF16 = ml_dtypes.bfloat16

# Problem sizes (hardcoded per the harness contract).
B_TOT, T, E, H = 512, 200, 128, 128
NCORES = 8
NSHARD = 4          # batch shards per direction
B = B_TOT // NSHARD  # 128 per core
P = 128
NG = 4
XCHUNK = 4          # x timesteps per input DMA
OCHUNK = 2          # out timesteps per output DMA

# Device gate order (g, f, i, o) -> Keras 4H order is (i, f, g, o).
KERAS_IDX = [2, 1, 0, 3]  # g, f, i, o
COL_G = slice(0 * B, 1 * B)
COL_F = slice(1 * B, 2 * B)
COL_I = slice(2 * B, 3 * B)
COL_O = slice(3 * B, 4 * B)
COL_FIO = slice(1 * B, 4 * B)


def _build_program(scalar_bias: float | None, t_steps: int = T):
    """Build the SPMD per-core Bass program (see module docstring)."""
    nc = bacc.Bacc("TRN2", target_bir_lowering=False, debug=False)

    # x chunked host-side as [T/XCHUNK, E, XCHUNK*B]; out written chunked
    # as [T/OCHUNK, H, OCHUNK*B] — so batched DMAs map 1:1 onto 2-d tiles.
    assert t_steps % XCHUNK == 0 and t_steps % OCHUNK == 0
    xT = nc.dram_tensor(
        "xT", [t_steps // XCHUNK, E, XCHUNK * B], BF16, kind="ExternalInput"
    ).ap()
    w = nc.dram_tensor("w", [2, NG, P, P], BF16, kind="ExternalInput").ap()
    u = nc.dram_tensor("u", [2, NG, P, P], BF16, kind="ExternalInput").ap()
    bias = nc.dram_tensor("bias", [2, NG, P, 1], FP32, kind="ExternalInput").ap()
    out = nc.dram_tensor(
        "out", [t_steps // OCHUNK, H, OCHUNK * B], FP32, kind="ExternalOutput"
    ).ap()

    with tile.TileContext(nc) as tc:
        with (
            tc.tile_pool(name="wpool", bufs=1) as wpool,
            tc.tile_pool(name="xpool", bufs=5) as xpool,
            tc.tile_pool(name="zg0pool", bufs=2, space="PSUM") as zg0pool,
            tc.tile_pool(name="z0pool", bufs=2, space="PSUM") as z0pool,
            tc.tile_pool(name="z1pool", bufs=2, space="PSUM") as z1pool,
            tc.tile_pool(name="gpool", bufs=3) as gpool,
            tc.tile_pool(name="tpool", bufs=3) as tpool,
            tc.tile_pool(name="cpool", bufs=3) as cpool,
            tc.tile_pool(name="hpool", bufs=4) as hpool,
            tc.tile_pool(name="opool", bufs=4) as opool,
        ):
            w_t: dict = {}
            u_t: dict = {}
            b_t: dict = {}
            for l in range(2):
                for g in range(NG):
                    wt = wpool.tile([P, P], BF16, tag=f"w{l}{g}")
                    nc.sync.dma_start(wt[:], w[l, g])
                    w_t[l, g] = wt
                    ut = wpool.tile([P, P], BF16, tag=f"u{l}{g}")
                    nc.sync.dma_start(ut[:], u[l, g])
                    u_t[l, g] = ut
                    if scalar_bias is None:
                        bt = wpool.tile([P, 1], FP32, tag=f"b{l}{g}")
                        nc.sync.dma_start(bt[:], bias[l, g])
                        b_t[l, g] = bt

            def bias_for(l, g):
                if scalar_bias is not None:
                    return float(scalar_bias)
                return b_t[l, g][:]

            # Layer-1 sigmoid-trick bias fix (scalar-bias fast path): its
            # g-gate is computed as 2*sigmoid(2*zg)-1 with host-doubled
            # weights, so it needs bias 2s while the single fused sigmoid
            # applies s; add the missing +s via a K=1 rank-1 matmul.
            if scalar_bias is not None:
                fix_lhs = wpool.tile([1, P], BF16, tag="fix_lhs")
                nc.vector.memset(fix_lhs[:], float(scalar_bias))
                fix_rhs = wpool.tile([1, B], BF16, tag="fix_rhs")
                nc.vector.memset(fix_rhs[:], 1.0)

            xtiles: dict = {}

            def load_x(t0):
                """DMA the XCHUNK-step x chunk starting at t0 into SBUF."""
                assert t0 % XCHUNK == 0
                xt = xpool.tile([P, XCHUNK * B], BF16, tag="xt")
                nc.sync.dma_start(xt[:], xT[t0 // XCHUNK])
                for k in range(XCHUNK):
                    xtiles[t0 + k] = (xt, k)

            def emit_x(t):
                """x-projection matmuls for step t. The g gate gets its
                OWN PSUM bank (own accumulation group, closed by U0_g
                alone) so tanh(zg) starts after the first U matmul; the
                f,i,o bank's group closes at the last U0 matmul. NOTE:
                concurrently-open groups must live in different banks —
                interleaved open groups within one bank corrupt PSUM.
                At t=0 there are no U0 matmuls (h(-1)=0): close here."""
                xt, k = xtiles.pop(t)
                rhs = xt[:, k * B : (k + 1) * B]
                zg = zg0pool.tile([P, NG * B], FP32, tag="zg0")
                nc.tensor.matmul(
                    zg[:, 0:B], lhsT=w_t[0, 0][:], rhs=rhs,
                    start=True, stop=(t == 0),
                )
                z0 = z0pool.tile([P, NG * B], FP32, tag="z0")
                for g in range(1, NG):
                    nc.tensor.matmul(
                        z0[:, g * B : (g + 1) * B],
                        lhsT=w_t[0, g][:], rhs=rhs,
                        start=(g == 1), stop=(t == 0 and g == NG - 1),
                    )
                return (zg, z0)

            def emit_u0(z0pair, h0_prev):
                """Recurrent matmuls; g first (closes its own bank's
                group, unblocking tanh_g immediately)."""
                zg, z0 = z0pair
                nc.tensor.matmul(
                    zg[:, 0:B], lhsT=u_t[0, 0][:], rhs=h0_prev[:],
                    start=False, stop=True,
                )
                for g in range(1, NG):
                    nc.tensor.matmul(
                        z0[:, g * B : (g + 1) * B],
                        lhsT=u_t[0, g][:], rhs=h0_prev[:],
                        start=False, stop=(g == NG - 1),
                    )

            def z1_fix_open():
                """Open z1(t)'s group with the dep-free +s bias-fix matmul
                (scalar-bias fast path). Being constant-input, it executes
                long before h0 is ready, keeping the 281ns fix off the
                critical W1->U1 stretch that gates sigma_all1. Cells not
                written by the opener are overwritten by their first
                in-group write (W1), same semantics the baseline relied
                on with start only on the group's first matmul."""
                z1 = z1pool.tile([P, NG * B], FP32, tag="z1")
                nc.tensor.matmul(
                    z1[:, COL_G], lhsT=fix_lhs[:], rhs=fix_rhs[:],
                    start=True, stop=False,
                )
                return z1

            def emit_w1_open(h0, close: bool, z1=None):
                """W1 @ h0(t) into z1(t). Opens the group unless the fix
                matmul already did. close=True when there is no U1 term
                (first step: h1(-1) = 0)."""
                opened = z1 is not None
                if not opened:
                    z1 = z1pool.tile([P, NG * B], FP32, tag="z1")
                for g in range(NG):
                    nc.tensor.matmul(
                        z1[:, g * B : (g + 1) * B],
                        lhsT=w_t[1, g][:], rhs=h0[:],
                        start=(g == 0 and not opened),
                        stop=(close and g == NG - 1),
                    )
                return z1

            def emit_u1_close(z1, h1_prev):
                """Close z1(t) with U1 @ h1(t-1); g first."""
                for g in range(NG):
                    nc.tensor.matmul(
                        z1[:, g * B : (g + 1) * B],
                        lhsT=u_t[1, g][:], rhs=h1_prev[:],
                        start=False, stop=(g == NG - 1),
                    )

            def gates_l0(z0pair):
                """Layer 0: g = tanh(zg + b_g) from its own bank (native
                Tanh — same ACT table as Sigmoid, no table reload), then
                f,i,o = sigmoid(z + b)."""
                zg, z0 = z0pair
                ys = gpool.tile([P, NG * B], BF16, tag="ys0")
                nc.scalar.activation(ys[:, COL_G], zg[:, 0:B],
                                     AF.Tanh, bias=bias_for(0, 0))
                if scalar_bias is not None:
                    nc.scalar.activation(ys[:, COL_FIO], z0[:, COL_FIO],
                                         AF.Sigmoid, bias=float(scalar_bias))
                else:
                    for g in range(1, NG):
                        nc.scalar.activation(
                            ys[:, g * B : (g + 1) * B],
                            z0[:, g * B : (g + 1) * B],
                            AF.Sigmoid, bias=bias_for(0, g),
                        )
                return ys

            def gates_l1(z1s):
                """Layer 1: sigmoid-trick — ONE fused sigmoid over all 4
                gates (g-gate weights host-doubled, +s fix matmul), then
                s = 2*sig-1 on DVE. Falls back to per-gate ops when the
                bias is not uniform."""
                ys = gpool.tile([P, NG * B], BF16, tag="ys1")
                if scalar_bias is not None:
                    nc.scalar.activation(ys[:, :], z1s[:, :],
                                         AF.Sigmoid, bias=float(scalar_bias))
                    s = tpool.tile([P, B], BF16, tag="s1")
                    nc.vector.tensor_scalar(
                        s[:], ys[:, COL_G], 2.0, -1.0,
                        mybir.AluOpType.mult, mybir.AluOpType.add,
                    )
                else:
                    nc.scalar.activation(ys[:, COL_G], z1s[:, COL_G],
                                         AF.Tanh, bias=bias_for(1, 0))
                    for g in range(1, NG):
                        nc.scalar.activation(
                            ys[:, g * B : (g + 1) * B],
                            z1s[:, g * B : (g + 1) * B],
                            AF.Sigmoid, bias=bias_for(1, g),
                        )
                    s = None
                return ys, s

            def emit_t1(l, ys, g_ap=None):
                # bf16 out keeps the DVE 2x perf mode (all operands 2-byte);
                # t1 = i*g is in (-1,1) and its inputs are already bf16.
                t1 = tpool.tile([P, B], BF16, tag=f"t1{l}")
                g_in = g_ap if g_ap is not None else ys[:, COL_G]
                nc.vector.tensor_mul(t1[:], ys[:, COL_I], g_in)
                return t1

            def emit_tfc(l, ys, c_prev):
                t2 = tpool.tile([P, B], FP32, tag=f"t2{l}")
                nc.gpsimd.tensor_mul(t2[:], ys[:, COL_F], c_prev[:])
                return t2

            def emit_c(l, t1, t2):
                c_new = cpool.tile([P, B], FP32, tag=f"c{l}")
                if t2 is None:
                    # first step: c = i*g; materialize as fp32 (GpSimd's
                    # f*c next step expects an fp32 c operand)
                    nc.vector.tensor_copy(c_new[:], t1[:])
                else:
                    nc.vector.tensor_add(c_new[:], t1[:], t2[:])
                return c_new

            def emit_tanh_c(l, c_new):
                tch = gpool.tile([P, B], BF16, tag=f"tc{l}")
                nc.scalar.activation(tch[:], c_new[:], AF.Tanh)
                return tch

            def emit_h(l, ys, tch):
                h_new = hpool.tile([P, B], BF16, tag=f"h{l}")
                nc.vector.tensor_mul(h_new[:], ys[:, COL_O], tch[:])
                return h_new

            ostage: dict = {}
            opending: list = []

            def flush_out():
                """Issue deferred out-DMAs (deps completed last iteration,
                so the GpSimd queue never head-of-line blocks on them)."""
                while opending:
                    row, ot = opending.pop(0)
                    # GpSimd queue: keeps stores off the Sync queue, whose
                    # head-of-line x-DMA WAR waits would delay them (and
                    # cascade into a DVE stall via the staging-tile WAR).
                    nc.gpsimd.dma_start(out[row], ot[:])

            def emit_out(t, h1t, h0t):
                """out(t) = h1(t) + h0(t), staged; DMA every OCHUNK."""
                base = (t // OCHUNK) * OCHUNK
                if t == base:
                    ostage[base] = opool.tile(
                        [P, OCHUNK * B], FP32, tag="ot", name="ot"
                    )
                ot = ostage[base]
                k = t - base
                nc.vector.tensor_add(ot[:, k * B : (k + 1) * B], h1t[:], h0t[:])
                if k == OCHUNK - 1:
                    opending.append((base // OCHUNK, ot))
                    del ostage[base]

            def l1_head(z1s, c1p):
                """Layer-1 gates + products for a step, given closed z1."""
                ys1, s1 = gates_l1(z1s)
                t1_1 = emit_t1(1, ys1, g_ap=(s1[:] if s1 is not None else None))
                tfc_1 = emit_tfc(1, ys1, c1p) if c1p is not None else None
                return ys1, t1_1, tfc_1

            # ---- software pipeline -------------------------------------
            # Iteration t computes layer-0 step t and layer-1 step t-1.
            c0_prev = None           # c0(t-1)
            c1_prev = None           # c1(t-2)
            h0_for_out: dict = {}    # h0(s) kept until out(s)
            z1_prev = None           # z1(t-1), closed by end of iter t-1

            load_x(0)
            if t_steps > XCHUNK:
                load_x(XCHUNK)
            z0 = emit_x(0)

            for t in range(t_steps):
                flush_out()
                # PE: dep-free work first — x-projection for t+1 and the
                # z1(t) bias-fix group opener.
                if t + 1 < t_steps:
                    nxt = t + 1 + XCHUNK
                    if (t + 1) % XCHUNK == 0 and nxt < t_steps:
                        load_x(nxt)
                    z0_next = emit_x(t + 1)
                else:
                    z0_next = None
                z1_pre = z1_fix_open() if scalar_bias is not None else None

                # --- layer 0, step t: critical chain head (ACT)
                ys0 = gates_l0(z0)
                t1_0 = emit_t1(0, ys0)
                tfc_0 = emit_tfc(0, ys0, c0_prev) if c0_prev is not None else None

                # --- layer 1, step t-1: ACT gap-fillers (z1(t-1) ready)
                if z1_prev is not None:
                    ys1, t1_1, tfc_1 = l1_head(z1_prev, c1_prev)
                else:
                    ys1 = None

                # --- layer 0 tail: c, tanh, h (the critical cycle)
                c0 = emit_c(0, t1_0, tfc_0)
                tc0 = emit_tanh_c(0, c0)
                h0 = emit_h(0, ys0, tc0)
                h0_for_out[t] = h0

                # --- PE: recurrent matmuls right behind h0
                if z0_next is not None:
                    emit_u0(z0_next, h0)
                z1 = emit_w1_open(h0, close=(t == 0), z1=z1_pre)

                # --- layer 1 tail for step t-1: c1, tanh, h1, out, U1
                if ys1 is not None:
                    c1 = emit_c(1, t1_1, tfc_1)
                    tc1 = emit_tanh_c(1, c1)
                    h1 = emit_h(1, ys1, tc1)
                    emit_out(t - 1, h1, h0_for_out.pop(t - 1))
                    emit_u1_close(z1, h1)
                    c1_prev = c1

                c0_prev = c0
                z1_prev = z1
                z0 = z0_next

            # ---- epilogue: layer-1 step T-1 ----------------------------
            ys1, t1_1, tfc_1 = l1_head(z1_prev, c1_prev)
            c1 = emit_c(1, t1_1, tfc_1)
            tc1 = emit_tanh_c(1, c1)
            h1 = emit_h(1, ys1, tc1)
            emit_out(t_steps - 1, h1, h0_for_out.pop(t_steps - 1))
            flush_out()

    nc.compile()
    return nc


_PROGRAM_CACHE: dict = {}


def _get_program(scalar_bias, t_steps: int = T):
    key = (scalar_bias, t_steps)
    if key not in _PROGRAM_CACHE:
        _PROGRAM_CACHE[key] = _build_program(scalar_bias, t_steps)
    return _PROGRAM_CACHE[key]


def _prep_inputs(x, W, U, b, scalar_bias):
    """Build the 8 per-core input maps."""
    in_maps = []
    per_dir = {}
    for d in range(2):
        wd = np.empty((2, NG, P, P), dtype=NP_BF16)
        ud = np.empty((2, NG, P, P), dtype=NP_BF16)
        bd = np.empty((2, NG, P, 1), dtype=np.float32)
        for l in range(2):
            for g in range(NG):
                ks = KERAS_IDX[g]
                # layer-1 candidate gate uses the sigmoid trick
                # tanh(z) = 2*sigmoid(2z) - 1: double its weights
                # (fast path only; +s bias fix is a device matmul)
                sc = 2.0 if (l == 1 and g == 0 and scalar_bias is not None) else 1.0
                wd[l, g] = (sc * W[l, d][:, ks * H : (ks + 1) * H]).astype(NP_BF16)
                ud[l, g] = (sc * U[l, d][:, ks * H : (ks + 1) * H]).astype(NP_BF16)
                bd[l, g, :, 0] = b[l, d][ks * H : (ks + 1) * H].astype(np.float32)
        per_dir[d] = (wd, ud, bd)

    for core in range(NCORES):
        d = core // NSHARD
        s = core % NSHARD
        xs = x[s * B : (s + 1) * B]           # [B, T, E]
        if d == 1:
            xs = xs[:, ::-1, :]               # time-reverse for backward dir
        xTc = np.transpose(xs, (1, 2, 0))     # [T, E, B]
        # chunk: [T/XC, XC, E, B] -> [T/XC, E, XC, B] -> [T/XC, E, XC*B]
        xTc = np.transpose(
            xTc.reshape(T // XCHUNK, XCHUNK, E, B), (0, 2, 1, 3)
        ).reshape(T // XCHUNK, E, XCHUNK * B)
        xTc = np.ascontiguousarray(xTc).astype(NP_BF16)
        wd, ud, bd = per_dir[d]
        in_maps.append({"xT": xTc, "w": wd, "u": ud, "bias": bd})
    return in_maps


def _unchunk_out(o):
    """[T/OC, H, OC*B] -> [T, H, B]"""
    o = o.reshape(T // OCHUNK, H, OCHUNK, B)
    return np.transpose(o, (0, 2, 1, 3)).reshape(T, H, B)


def _postprocess(results, dtype):
    full = np.empty((B_TOT, T, H), dtype=np.float32)
    for s in range(NSHARD):
        fw = _unchunk_out(np.asarray(results[s]["out"]))           # [T, H, B]
        bw = _unchunk_out(np.asarray(results[NSHARD + s]["out"]))  # reversed t
        fw_b = np.transpose(fw, (2, 0, 1))            # [B, T, H]
        bw_b = np.transpose(bw, (2, 0, 1))[:, ::-1, :]
        full[s * B : (s + 1) * B] = (fw_b + bw_b) * 0.5
    return full.astype(dtype)


def run(x, W, U, b, **spmd_kwargs):
    """Run the kernel; returns (output, BassKernelResults)."""
    x = np.asarray(x)
    W = np.asarray(W)
    U = np.asarray(U)
    b = np.asarray(b)
    b0 = float(np.asarray(b).flat[0])
    scalar_bias = b0 if np.all(b == b0) else None
    nc = _get_program(scalar_bias)
    in_maps = _prep_inputs(x, W, U, b, scalar_bias)
    res = run_bass_kernel_spmd(nc, in_maps, core_ids=list(range(NCORES)), **spmd_kwargs)
    out = _postprocess(res.results, x.dtype)
    return out, res


def kernel(x, W, U, b):
    out, _ = run(x, W, U, b)
    return out
